# revision 29
# baseline (speedup 1.0000x reference)
"""CBOW negative-sampling loss kernel for 8 TRN2 NeuronCores.

Strategy (data-parallel, per sharding hint):
  - Shard the batch (B=16384) across 8 cores -> 2048 rows/core.
  - Per core the embedding tables are COMPACTED on host: only the
    distinct rows this core's lookups touch (<=20480 for in_emb,
    <=22528 for out_emb) are uploaded, relabelled 0..n-1, bf16.
    Indices fit int16.
  - ALL 43008 row-gathers per core go through batched dma_gather
    ucode in 11 large chunks (4096/2048 indices each) spread
    round-robin over 4 SWDGE queues.  Measured on HW: 4-queue
    dma_gather sustains the full 43008-row / 11 MB gather in ~30 us
    (HBM roofline), vs ~285 us single-queue (descriptor-gen bound)
    and ~440 us for the per-128-row indirect_dma_start path the
    previous version used.
  - Gather lists are SLOT-MAJOR: ctx chunk c carries context slots
    {2c, 2c+1} for all 16 tiles; tn chunk c carries candidates
    {2c, 2c+1} (candidate 0 = target).  Block layout per chunk is
    [local_slot(2) x tile(16) x dim(128)], so the context-sum tree
    and the per-candidate score reductions each run as a handful of
    2048..4096-column DVE instructions (no per-tile small ops).
  - DVE computes the 10-way context sum with a 9-add binary tree in
    bf16, then per candidate k one 2048-col multiply + one segmented
    f32 reduce -> s_all[P, (k,t)].
  - ACT applies sigmoid to ALL scores with scale -0.1 (folds the
    1/10 context-mean normalisation and the negation) and a single
    ln(+eps) with free-dim accumulation.  The target's positive term
    is recovered on host via log sig(x) - log sig(-x) = x, i.e.
    loss row-sum = sum_c log(sig(-s_c/10)+eps) + s_pos/10.
"""

import os

import numpy as np

import concourse.bacc as bacc
import concourse.bass as bass
import concourse.mybir as mybir
import concourse.tile as tile
from concourse.bass_utils import run_bass_kernel_spmd

VOCAB = 100000
DIM = 128
B = 16384
CWIN = 10
K = 10
EPS = 1e-9
NCORES = 8
P = 128
BPC = B // NCORES            # 2048 batch rows per core
NTILES = BPC // P            # 16 tiles of 128 rows
CT_IN = 20480                # compacted in_emb rows (= 2048*10 worst case)
CT_OUT = 22528               # compacted out_emb rows (= 2048*11 worst case)
NQUEUES = 4

CTX_N = BPC * CWIN           # 20480 ctx lookups per core
TN_N = BPC * (K + 1)         # 22528 target+negative lookups per core
# gather chunking in units of SLOTS (1 slot = 2048 rows = all 16 tiles
# of one context position / candidate): (first_slot, nslots)
CTX_CH = [(0, 2), (2, 2), (4, 2), (6, 2), (8, 2)]
TN_CH = [(0, 2), (2, 2), (4, 2), (6, 2), (8, 2), (10, 1)]

F32 = mybir.dt.float32
BF16 = mybir.dt.bfloat16
I16 = mybir.dt.int16
MULT = mybir.AluOpType.mult
ADD = mybir.AluOpType.add
AX_X = mybir.AxisListType.X
SIGMOID = mybir.ActivationFunctionType.Sigmoid
LN = mybir.ActivationFunctionType.Ln


def build_kernel_body(tc, ctxidx, tnidx, ctab_in, ctab_out, usum, R=1):
    """Emit the per-core program.

    ctxidx: [P, 1280] int16 wrapped dma_gather lists; chunk (s0, ns)
            of CTX_CH covers context slots s0..s0+ns-1 of all 16
            tiles: list position j'*2048 + t*128 + p  ->
            cin[p, t, s0+j'].
    tnidx:  [P, 1408] int16; TN_CH chunks cover candidates the same
            way from ctab_out; candidate 0 is the target.
    usum:   [P, 4] f32; cols 0/1 = per-k-batch sums of
            log(sig(-s/10)+eps); col 2 = sum over tiles of raw
            target score (context-SUM dot target, no 1/10); col 3
            unused (stays zero).
    """
    nc = tc.nc
    with (
        tc.tile_pool(name="io", bufs=1) as io_pool,
        tc.tile_pool(name="g", bufs=1) as gpool,
        tc.tile_pool(name="w", bufs=1) as wpool,
    ):
        ctxidx_t = io_pool.tile([P, CTX_N // 16], I16)
        nc.sync.dma_start(out=ctxidx_t[:], in_=ctxidx[:, :])
        tnidx_t = io_pool.tile([P, (TN_N // 16)], I16)
        nc.sync.dma_start(out=tnidx_t[:], in_=tnidx[:, :])

        eps_t = io_pool.tile([P, 1], F32)
        nc.vector.memset(eps_t[:], EPS)

        # warm the ACT function tables (sigmoid + ln) so the two
        # ~1.3us table loads overlap the gather phase instead of
        # sitting on the critical tail
        warm = io_pool.tile([P, 1], F32)
        nc.scalar.activation(warm[:], eps_t[:], SIGMOID)
        nc.scalar.activation(warm[:], eps_t[:], LN, bias=eps_t[:])

        s_all = io_pool.tile([P, NTILES * (K + 1)], F32)
        us = io_pool.tile([P, 4], F32)

        W = NTILES * DIM  # 2048 cols per slot
        for r in range(R):
            # ---- gathers: 11 chunks, round-robin over 4 queues -----
            # (4096-idx chunks measured faster than 8192: smaller
            # tiles pipeline better against their DVE consumers)
            q = 0
            ctx_g, tn_g = [], []
            for pre, chunks, tab, idx_t, tiles in (
                ("gc", CTX_CH, ctab_in, ctxidx_t, ctx_g),
                ("gt", TN_CH, ctab_out, tnidx_t, tn_g),
            ):
                for (s0, ns) in chunks:
                    n = ns * BPC
                    t = gpool.tile([P, ns * W], BF16, tag=f"{pre}{s0}")
                    nc.gpsimd.dma_gather(
                        out_ap=t[:].rearrange("p (q d) -> p q d", d=DIM),
                        in_ap=tab[:, :],
                        idxs_ap=idx_t[:, s0 * (BPC // 16):
                                      (s0 + ns) * (BPC // 16)],
                        num_idxs=n,
                        num_idxs_reg=n,
                        elem_size=DIM,
                        single_packet=False,
                        queue_num=q % NQUEUES,
                    )
                    q += 1
                    tiles.append(t)

            def slot(tiles, chunks, s):
                for (s0, ns), t in zip(chunks, tiles):
                    if s0 <= s < s0 + ns:
                        return t[:, (s - s0) * W:(s - s0 + 1) * W]
                raise AssertionError(s)

            if os.environ.get("KCFG_BARRIER", "0") == "1":
                # A/B experiment: serialize all gathers before any DVE
                # work (tests whether DVE<->GpSimd SBUF port contention
                # during descriptor generation costs more than the
                # lost overlap)
                bar = wpool.tile([P, 1], BF16, tag="bar")
                nc.vector.tensor_add(
                    bar[:], tn_g[-1][:, 0:1], ctx_g[-1][:, 0:1])

            # ---- context-sum tree (9 adds, bf16, big spans) ----
            t1 = []
            for j in range(5):
                t = wpool.tile([P, W], BF16, tag=f"t1{j}")
                nc.vector.tensor_add(
                    t[:], slot(ctx_g, CTX_CH, 2 * j),
                    slot(ctx_g, CTX_CH, 2 * j + 1),
                )
                t1.append(t)
            t2a = wpool.tile([P, W], BF16, tag="t2a")
            nc.vector.tensor_add(t2a[:], t1[0][:], t1[1][:])
            t2b = wpool.tile([P, W], BF16, tag="t2b")
            nc.vector.tensor_add(t2b[:], t1[2][:], t1[3][:])
            cs = wpool.tile([P, W], BF16, tag="cs")
            nc.vector.tensor_add(cs[:], t2a[:], t2b[:])
            nc.vector.tensor_add(cs[:], cs[:], t1[4][:])
            cs3 = cs[:].rearrange("p (t d) -> p t d", d=DIM)

            # ---- scores: two k-batches; per batch: per-k 2048-col mult
            # into a shared product buffer, then 3 levels of 2x-eligible
            # interleaved fold-adds halving the per-(k,t) dim width
            # 128 -> 16, then one short 1x segmented reduce.
            KB = [list(range(0, 6)), list(range(6, K + 1))]
            prod = wpool.tile([P, 6 * W], BF16, tag="prod")
            f1 = wpool.tile([P, 6 * W // 2], BF16, tag="f1")
            f2 = wpool.tile([P, 6 * W // 4], BF16, tag="f2")
            f3 = wpool.tile([P, 6 * W // 8], BF16, tag="f3")
            for kb in KB:
                nk = len(kb)
                for i, k in enumerate(kb):
                    src = slot(tn_g, TN_CH, k)
                    nc.vector.tensor_tensor(
                        prod[:, i * W:(i + 1) * W].rearrange(
                            "p (t d) -> p t d", d=DIM),
                        src.rearrange("p (t d) -> p t d", d=DIM),
                        cs3, MULT,
                    )
                # fold 128 -> 64 -> 32 -> 16 per (k, t) block
                for (src_t, dst_t, w) in (
                    (prod, f1, DIM), (f1, f2, DIM // 2), (f2, f3, DIM // 4),
                ):
                    s4 = src_t[:, 0:nk * NTILES * w].rearrange(
                        "p (b two d) -> p b two d", two=2, d=w // 2)
                    d4 = dst_t[:, 0:nk * NTILES * (w // 2)].rearrange(
                        "p (b one d) -> p b one d", one=1, d=w // 2)
                    nc.vector.tensor_add(d4, s4[:, :, 0:1], s4[:, :, 1:2])
                sl = s_all[:, kb[0] * NTILES:(kb[-1] + 1) * NTILES]
                nc.vector.tensor_reduce(
                    out=sl,
                    in_=f3[:, 0:nk * NTILES * (DIM // 8)].rearrange(
                        "p (s d) -> p s d", d=DIM // 8),
                    axis=AX_X, op=ADD,
                )
                if kb[0] == 0:
                    # target raw-score row sum (candidate 0 lives in
                    # the first batch)
                    nc.vector.tensor_reduce(
                        out=us[:, 2:3], in_=s_all[:, 0:NTILES],
                        axis=AX_X, op=ADD,
                    )
                # per-batch activation chain overlaps the other
                # batch's DVE work (tables already warm)
                b = 0 if kb[0] == 0 else 1
                nw = nk * NTILES
                sig = io_pool.tile([P, 6 * NTILES], F32, tag=f"sig{b}")
                nc.scalar.activation(sig[:, 0:nw], sl, SIGMOID, scale=-0.1)
                lnv = io_pool.tile([P, 6 * NTILES], F32, tag=f"lnv{b}")
                nc.scalar.activation(
                    lnv[:, 0:nw], sig[:, 0:nw], LN, bias=eps_t[:],
                    accum_out=us[:, b:b + 1],
                )

        nc.sync.dma_start(out=usum[:, :], in_=us[:])


def build_nc(R=1):
    nc = bacc.Bacc(
        "TRN2",
        target_bir_lowering=False,
        debug=False,
        enable_asserts=False,
        num_devices=NCORES,
        num_swdge_queues=NQUEUES,
    )
    ctxidx = nc.dram_tensor(
        "ctxidx", [P, CTX_N // 16], I16, kind="ExternalInput"
    )
    tnidx = nc.dram_tensor(
        "tnidx", [P, TN_N // 16], I16, kind="ExternalInput"
    )
    ctab_in = nc.dram_tensor("ctab_in", [CT_IN, DIM], BF16,
                             kind="ExternalInput")
    ctab_out = nc.dram_tensor("ctab_out", [CT_OUT, DIM], BF16,
                              kind="ExternalInput")
    usum = nc.dram_tensor("usum", [P, 4], F32, kind="ExternalOutput")
    with tile.TileContext(nc) as tc:
        build_kernel_body(tc, ctxidx.ap(), tnidx.ap(), ctab_in.ap(),
                          ctab_out.ap(), usum.ap(), R=R)
    nc.compile()
    return nc


def _wrap16(arr):
    """flat index list -> [128, n/16] int16 dma_gather layout."""
    w = np.asarray(arr).reshape(-1, 16).T
    return np.tile(w, (8, 1)).astype(np.int16)


def make_in_maps(context, target, negatives, in_emb, out_emb):
    context = np.asarray(context).astype(np.int64)
    target = np.asarray(target).astype(np.int64)
    negatives = np.asarray(negatives).astype(np.int64)
    in_emb = np.asarray(in_emb, dtype=np.float32)
    out_emb = np.asarray(out_emb, dtype=np.float32)
    bf16 = mybir.dt.np(BF16)
    tn_full = np.concatenate([target[:, None], negatives], axis=1)  # [B, 11]
    in_maps = []
    for c in range(NCORES):
        sl = slice(c * BPC, (c + 1) * BPC)
        # [P, NTILES, slots] index cubes (partition = batch row % 128)
        ctx_t = (
            context[sl].reshape(NTILES, P, CWIN).transpose(1, 0, 2)
        )  # [P, T, 10]
        tn_t = (
            tn_full[sl].reshape(NTILES, P, K + 1).transpose(1, 0, 2)
        )  # [P, T, 11], slot 0 = target

        # per-(core, table) compaction
        uin, cin = np.unique(ctx_t, return_inverse=True)
        cin = cin.reshape(ctx_t.shape).astype(np.int32)
        uout, ctn = np.unique(tn_t, return_inverse=True)
        ctn = ctn.reshape(tn_t.shape).astype(np.int32)
        assert len(uin) <= CT_IN and len(uout) <= CT_OUT
        ctab_in = np.zeros((CT_IN, DIM), dtype=bf16)
        ctab_in[: len(uin)] = in_emb[uin].astype(bf16)
        ctab_out = np.zeros((CT_OUT, DIM), dtype=bf16)
        ctab_out[: len(uout)] = out_emb[uout].astype(bf16)

        # slot-major gather lists: position j*2048 + t*128 + p,
        # wrapped per gather chunk
        ctx_list = cin.transpose(2, 1, 0).reshape(-1)   # [20480]
        tn_list = ctn.transpose(2, 1, 0).reshape(-1)    # [22528]
        ctxidx = np.concatenate(
            [_wrap16(ctx_list[s0 * BPC:(s0 + ns) * BPC])
             for (s0, ns) in CTX_CH], axis=1)
        tnidx = np.concatenate(
            [_wrap16(tn_list[s0 * BPC:(s0 + ns) * BPC])
             for (s0, ns) in TN_CH], axis=1)

        in_maps.append(
            {
                "ctxidx": np.ascontiguousarray(ctxidx),
                "tnidx": np.ascontiguousarray(tnidx),
                "ctab_in": ctab_in,
                "ctab_out": ctab_out,
            }
        )
    return in_maps


_NC_CACHE = []
LAST_RESULT = None  # BassKernelResults of the most recent run (for profiling)


def kernel(**inputs) -> np.ndarray:
    global LAST_RESULT
    in_maps = make_in_maps(
        inputs["context"],
        inputs["target"],
        inputs["negatives"],
        inputs["in_emb"],
        inputs["out_emb"],
    )
    if not _NC_CACHE:
        _NC_CACHE.append(build_nc())
    nc = _NC_CACHE[0]
    res = run_bass_kernel_spmd(nc, in_maps, core_ids=list(range(NCORES)))
    LAST_RESULT = res
    total = 0.0
    for r in res.results:
        u = r["usum"].astype(np.float64)
        total += u[:, 0].sum() + u[:, 1].sum() + 0.1 * u[:, 2].sum()
    return np.array(-total / B, dtype=np.float32)
